# revision 37
# baseline (speedup 1.0000x reference)
# Trainium2 Bass kernel for a causal multi-head attention block.
#
# Reference computation (fp32):
#   qkv = x @ w_attn + b_attn ; split into q,k,v heads (N=16, H=64)
#   scores = q @ k^T / sqrt(H), causal mask, softmax over keys
#   out = (weights @ v) reshaped, then out @ w_proj + b_proj
#
# Sharding: 8 cores = 2 batches x 4 head-groups (4 heads each).
#   - batch data-parallel, heads tensor-parallel (c_attn columns / c_proj rows)
#   - each core emits a partial [T, D] projection output; host sums the 4
#     head-group partials per batch and adds b_proj (the gather step).
#
# On-device layout trick: scores are computed TRANSPOSED (S^T[s,t]) so that
# exp(S^T) tiles are directly usable as the stationary operand of the
# weights@V matmul (contraction over s = partition dim), eliminating all
# softmax-weight transposes.  Row sums come free via a ones-column in V.

import math

import numpy as np

B, T, D = 2, 2048, 1024
NHEAD, H = 16, 64
HPC = 4            # heads per core
CD = HPC * H       # 256 head-dim columns per core
N_CORES = 8
P = 128            # partitions
TT = T // P        # 16 t-tiles of 128
TB = T // 512      # 4 t-blocks of 512
KD = D // P        # 8 contraction tiles over D

_CACHE = {}


def _build_module(mm_dt_name: str):
    import contextlib

    import concourse.bass as bass  # noqa: F401
    import concourse.mybir as mybir
    import concourse.tile as tile
    from concourse import bacc

    f32 = mybir.dt.float32
    mdt = getattr(mybir.dt, mm_dt_name)
    # dtype for the softmax-weight @ V matmul operands: bf16 runs at
    # 1 cycle/row for any free-dim (fp32r pays 4x below N=256) and gets
    # fast weight loads on HW.  The row-sum is computed from the same
    # bf16 weights, so normalization cancels most of the rounding error.
    avdt = mybir.dt.bfloat16 if mm_dt_name == "float32r" else mdt

    nc = bacc.Bacc("TRN2", target_bir_lowering=False, debug=False)

    x_d = nc.dram_tensor("x", [T, D], mdt, kind="ExternalInput").ap()
    wqk_d = nc.dram_tensor("wqk", [D, 2 * CD], mdt, kind="ExternalInput").ap()
    bqk_d = nc.dram_tensor("bqk", [1, 2 * CD], mdt, kind="ExternalInput").ap()
    wv_d = nc.dram_tensor("wv", [D, CD], mdt, kind="ExternalInput").ap()
    bv_d = nc.dram_tensor("bv", [1, CD], mdt, kind="ExternalInput").ap()
    wp_d = nc.dram_tensor("wp", [CD, D], mdt, kind="ExternalInput").ap()
    ident_d = nc.dram_tensor("ident", [P, P], mdt, kind="ExternalInput").ap()
    mask_d = nc.dram_tensor("mask", [P, P], avdt, kind="ExternalInput").ap()
    ones_d = nc.dram_tensor("ones", [1, 512], mdt, kind="ExternalInput").ap()
    onescol_d = nc.dram_tensor("onescol", [P, 2 * HPC], avdt, kind="ExternalInput").ap()
    y_d = nc.dram_tensor("y", [T, D], f32, kind="ExternalOutput").ap()

    with tile.TileContext(nc) as tc, contextlib.ExitStack() as ctx:
        const_p = ctx.enter_context(tc.tile_pool(name="const", bufs=1))
        w_p = ctx.enter_context(tc.tile_pool(name="weights", bufs=1))
        x_p = ctx.enter_context(tc.tile_pool(name="xin", bufs=8))
        xt_p = ctx.enter_context(tc.tile_pool(name="xt", bufs=2))
        qkt_p = ctx.enter_context(tc.tile_pool(name="qkt", bufs=1))
        v_p = ctx.enter_context(tc.tile_pool(name="vbuf", bufs=1))
        e_p = ctx.enter_context(tc.tile_pool(name="epool", bufs=16))
        attn_p = ctx.enter_context(tc.tile_pool(name="attn", bufs=1))
        small_p = ctx.enter_context(tc.tile_pool(name="small", bufs=8))
        # single PSUM pool, 8 banks total:
        #   wps   [128,512]x2  (phase1 transposes/qkT/V + proj)      2 banks
        #   sp    [128,1024]x2 (scores)                              4 banks
        #   accp* [128,264]x2  (AV accumulators, 4 groups per bank)  2 banks
        psp = ctx.enter_context(tc.tile_pool(name="psp", bufs=2, space="PSUM"))

        # ---- loads, ordered by when phase 1 needs them, spread across
        # the three DMA-capable engines' queues ----
        x_dma_engines = [nc.sync, nc.scalar, nc.gpsimd]
        rr = [0]

        def load(tile_ap, dram_ap):
            x_dma_engines[rr[0] % 3].dma_start(tile_ap, dram_ap)
            rr[0] += 1

        ident = const_p.tile([P, P], mdt, name="ident_sb")
        nc.sync.dma_start(ident, ident_d)

        x_sb = {}
        for g in range(4):
            xt_in = x_p.tile([P, D], mdt, name="x_sb", tag="x_sb")
            load(xt_in, x_d[P * g : P * (g + 1), :])
            x_sb[g] = xt_in

        wqk_sb = []
        for k in range(KD):
            t = w_p.tile([P, 2 * CD], mdt, name=f"wqk{k}", tag=f"wqk{k}")
            nc.sync.dma_start(t, wqk_d[P * k : P * (k + 1), :])
            wqk_sb.append(t)
        bqk = const_p.tile([1, 2 * CD], mdt, name="bqk_sb")
        nc.sync.dma_start(bqk, bqk_d)
        ones = const_p.tile([1, 512], mdt, name="ones_sb")
        nc.sync.dma_start(ones, ones_d)
        wv_sb = []
        for k in range(KD):
            t = w_p.tile([P, CD], mdt, name=f"wv{k}", tag=f"wv{k}")
            nc.sync.dma_start(t, wv_d[P * k : P * (k + 1), :])
            wv_sb.append(t)
        bv = const_p.tile([1, CD], mdt, name="bv_sb")
        nc.sync.dma_start(bv, bv_d)
        onescol = const_p.tile([P, 2 * HPC], avdt, name="onescol_sb")
        nc.sync.dma_start(onescol, onescol_d)
        mask = const_p.tile([P, P], avdt, name="mask_sb")
        nc.sync.dma_start(mask, mask_d)
        for g in range(4, 8):
            xt_in = x_p.tile([P, D], mdt, name="x_sb", tag="x_sb")
            load(xt_in, x_d[P * g : P * (g + 1), :])
            x_sb[g] = xt_in
        wp_sb = []
        for c in range(CD // P):
            t = w_p.tile([P, D], mdt, name=f"wp{c}", tag=f"wp{c}")
            nc.sync.dma_start(t, wp_d[P * c : P * (c + 1), :])
            wp_sb.append(t)

        # persistent activation buffers
        qkt_sb = {}
        for m in range(4):
            for j in range(TB):
                qkt_sb[(m, j)] = qkt_p.tile(
                    [P, 512], mdt, name=f"qkt{m}_{j}", tag=f"qkt{m}_{j}"
                )
        v_sb = []
        for i in range(TT):
            v_sb.append(
                v_p.tile([P, HPC * (H + 2)], avdt, name=f"v{i}", tag=f"v{i}")
            )
        attn_t = [
            attn_p.tile([P, 4 * CD], mdt, name=f"attn{c}", tag=f"attn{c}")
            for c in range(TB)
        ]
        G = H + 2

        def phase1_block(j):
            """x^T transposes + qk^T + V for 512-wide t-block j."""
            for ti in range(4):
                g = 4 * j + ti
                if g in x_sb:
                    continue
                xt_in = x_p.tile([P, D], mdt, name="x_sb", tag="x_sb")
                load(xt_in, x_d[P * g : P * (g + 1), :])
                x_sb[g] = xt_in
            xt_blk = []
            for k in range(KD):
                pt = psp.tile([P, 512], mdt, name="xtp", tag="wps")
                for ti in range(4):
                    nc.tensor.transpose(
                        pt[:, P * ti : P * (ti + 1)],
                        x_sb[4 * j + ti][:, P * k : P * (k + 1)],
                        ident,
                    )
                xt = xt_p.tile([P, 512], mdt, name=f"xt{k}", tag=f"xt{k}")
                nc.vector.tensor_copy(xt, pt)
                xt_blk.append(xt)

            for m in range(4):
                ps = psp.tile([P, 512], f32, name="qkp", tag="wps")
                nc.tensor.matmul(
                    ps,
                    bqk[0:1, P * m : P * (m + 1)],
                    ones[0:1, 0:512],
                    start=True,
                    stop=False,
                )
                for k in range(KD):
                    nc.tensor.matmul(
                        ps,
                        wqk_sb[k][:, P * m : P * (m + 1)],
                        xt_blk[k],
                        start=False,
                        stop=(k == KD - 1),
                    )
                nc.scalar.copy(qkt_sb[(m, j)], ps)

            for ti in range(4):
                g = 4 * j + ti
                ps = psp.tile([P, CD], f32, name="vp", tag="wps")
                nc.tensor.matmul(
                    ps,
                    ones[0:1, 0:P],
                    bv[0:1, :],
                    start=True,
                    stop=False,
                )
                for k in range(KD):
                    nc.tensor.matmul(
                        ps,
                        xt_blk[k][:, P * ti : P * (ti + 1)],
                        wv_sb[k],
                        start=False,
                        stop=(k == KD - 1),
                    )
                vg = v_sb[g].rearrange("p (g c) -> p g c", g=HPC)
                nc.vector.tensor_copy(
                    vg[:, :, 0:H], ps.rearrange("p (g c) -> p g c", g=HPC)
                )
                nc.vector.tensor_copy(
                    vg[:, :, H : H + 2],
                    onescol.rearrange("p (g c) -> p g c", c=2),
                )

        def attention(tb):
            """S^T -> exp -> AV for 512-wide t-block tb, heads processed in
            pairs (partition bases 0 and 64) so the two K=64 score matmuls
            occupy disjoint PE row groups and run concurrently; one psum
            tile holds both heads' scores so a single exp covers both."""
            for hp in range(2):
                h0, h1 = 2 * hp, 2 * hp + 1
                mq, mk = hp, 2 + hp
                acc_t = [
                    psp.tile([P, 4 * 66], f32, name="accp", tag=f"accp{a}",
                             bufs=1)
                    for a in range(2)
                ]
                n_s = 4 * tb + 4  # s-tiles 0 .. 4*tb+3
                for i in range(n_s):
                    first = max(0, i - 4 * tb)  # first valid jj in block
                    sps = psp.tile([P, 1024], f32, name="sp", tag="sp",
                                   bufs=2)
                    # trim fully-masked leading columns when it helps:
                    # fp32r matmuls below N=256 run at 1/4 rate, so only
                    # slice when the remaining width stays >= 256.
                    c0 = P * first if 512 - P * first >= 256 else 0
                    for hh, pb in ((0, 0), (1, 64)):
                        nc.tensor.matmul(
                            sps[:, 512 * hh + c0 : 512 * hh + 512],
                            qkt_sb[(mk, i // 4)][
                                pb : pb + H, P * (i % 4) : P * (i % 4 + 1)
                            ],
                            qkt_sb[(mq, tb)][pb : pb + H, c0:512],
                            start=True,
                            stop=True,
                        )
                    et = e_p.tile([P, 1024], avdt, name="et", tag="et")
                    if first:
                        w = 512 - P * first
                        nc.scalar.activation(
                            et.rearrange("p (g c) -> p g c", g=2)[
                                :, :, P * first : 512
                            ],
                            sps.rearrange("p (g c) -> p g c", g=2)[
                                :, :, P * first : 512
                            ],
                            mybir.ActivationFunctionType.Exp,
                            scale=1.0 / math.sqrt(H),
                        )
                    else:
                        nc.scalar.activation(
                            et,
                            sps,
                            mybir.ActivationFunctionType.Exp,
                            scale=1.0 / math.sqrt(H),
                        )
                    if 0 <= i - 4 * tb <= 3:
                        jj = i - 4 * tb
                        for hh in range(2):
                            nc.vector.tensor_mul(
                                et[:, 512 * hh + P * jj : 512 * hh + P * (jj + 1)],
                                et[:, 512 * hh + P * jj : 512 * hh + P * (jj + 1)],
                                mask,
                            )
                    for jj in range(first, 4):
                        jglob = 4 * tb + jj
                        for hh in range(2):
                            # start=True clears has_written for the WHOLE
                            # psum bank: only the first group per bank
                            # issues it.
                            nc.tensor.matmul(
                                acc_t[hh][:, 66 * jj : 66 * jj + 66],
                                et[:, 512 * hh + P * jj : 512 * hh + P * (jj + 1)],
                                v_sb[i][:, G * (h0 + hh) : G * (h0 + hh) + 66],
                                start=(i == 0 and jj == 0),
                                stop=(i == jglob),
                                skip_group_check=True,
                            )
                for jj in range(4):
                    for hh in range(2):
                        h = h0 + hh
                        s0 = 66 * jj
                        rec = small_p.tile([P, 1], f32, name="rec", tag="rec")
                        nc.vector.reciprocal(
                            rec, acc_t[hh][:, s0 + H : s0 + H + 1]
                        )
                        nc.vector.tensor_scalar_mul(
                            attn_t[tb][:, CD * jj + H * h : CD * jj + H * (h + 1)],
                            acc_t[hh][:, s0 : s0 + H],
                            rec,
                        )

        def projection(jb):
            """attn^T transposes + y = attn @ wp for 512-wide t-block jb."""
            attnT = {}
            for c in range(CD // P):
                pt = psp.tile([P, 512], mdt, name="atp", tag="wps")
                for ti in range(4):
                    nc.tensor.transpose(
                        pt[:, P * ti : P * (ti + 1)],
                        attn_t[jb][:, CD * ti + P * c : CD * ti + P * (c + 1)],
                        ident,
                    )
                at = e_p.tile([P, 512], mdt, name="at", tag="at", bufs=4)
                if jb == 3 and c == 0:
                    nc.scalar.copy(at, pt)
                else:
                    nc.vector.tensor_copy(at, pt)
                attnT[c] = at

            for jl in range(4):
                jt = 4 * jb + jl
                for n in range(2):
                    # block 3's projection runs after all attention: the
                    # score psum slots are free then
                    ps = psp.tile([P, 512], f32, name="yp",
                                  tag=("sp" if jb == 3 else "wps"))
                    for c in range(CD // P):
                        nc.tensor.matmul(
                            ps,
                            attnT[c][:, P * jl : P * (jl + 1)],
                            wp_sb[c][:, 512 * n : 512 * (n + 1)],
                            start=(c == 0),
                            stop=(c == CD // P - 1),
                        )
                    ysb = small_p.tile([P, 512], f32, name="ysb", tag="ysb",
                                       bufs=4)
                    if jb == 3 and (jl + n) % 2 == 0:
                        nc.scalar.copy(ysb, ps)
                    else:
                        nc.vector.tensor_copy(ysb, ps)
                    (nc.sync if (jb < 3 or n == 0) else nc.scalar).dma_start(
                        y_d[P * jt : P * (jt + 1), 512 * n : 512 * (n + 1)],
                        ysb,
                    )

        # emission order chosen so chunk-0 attention (ACT-bound) can overlap
        # the second half of phase 1 (PE-bound), and each chunk's projection
        # overlaps the next chunk's attention.
        phase1_block(0)
        attention(0)
        phase1_block(1)
        attention(1)
        phase1_block(2)
        projection(0)
        attention(2)
        phase1_block(3)
        projection(1)
        attention(3)
        projection(2)
        projection(3)

    nc.compile()
    return nc


def _get_module(mm_dt_name: str):
    if mm_dt_name not in _CACHE:
        _CACHE[mm_dt_name] = _build_module(mm_dt_name)
    return _CACHE[mm_dt_name]


def kernel(x, w_attn, b_attn, w_proj, b_proj, mm_dt_name: str = "float32r",
           trace: bool = False):
    from concourse.bass_utils import run_bass_kernel_spmd

    x = np.asarray(x, dtype=np.float32)
    w_attn = np.asarray(w_attn, dtype=np.float32)
    b_attn = np.asarray(b_attn, dtype=np.float32)
    w_proj = np.asarray(w_proj, dtype=np.float32)
    b_proj = np.asarray(b_proj, dtype=np.float32)

    nc = _get_module(mm_dt_name)

    import ml_dtypes

    avnp = np.dtype(ml_dtypes.bfloat16) if mm_dt_name == "float32r" else np.float32
    ident = np.eye(P, dtype=np.float32)
    mask = np.triu(np.ones((P, P), dtype=avnp))
    ones = np.ones((1, 512), dtype=np.float32)

    in_maps = []
    for core in range(N_CORES):
        b = core // 4
        g = core % 4
        c0 = CD * g
        wq = w_attn[:, c0 : c0 + CD]
        wk = w_attn[:, D + c0 : D + c0 + CD]
        wv = w_attn[:, 2 * D + c0 : 2 * D + c0 + CD]
        bq = b_attn[c0 : c0 + CD]
        bk = b_attn[D + c0 : D + c0 + CD]
        bvv = b_attn[2 * D + c0 : 2 * D + c0 + CD]
        in_maps.append(
            {
                "x": np.ascontiguousarray(x[b]),
                "wqk": np.ascontiguousarray(np.concatenate([wq, wk], axis=1)),
                "bqk": np.concatenate([bq, bk])[None, :].copy(),
                "wv": np.ascontiguousarray(wv),
                "bv": bvv[None, :].copy(),
                "wp": np.ascontiguousarray(w_proj[c0 : c0 + CD, :]),
                "ident": ident,
                "mask": mask,
                "ones": ones,
                "onescol": np.tile(np.array([1.0, 0.0], avnp), (P, HPC)),
            }
        )

    res = run_bass_kernel_spmd(
        nc, in_maps, core_ids=list(range(N_CORES)), trace=trace
    )

    out = np.zeros((B, T, D), dtype=np.float32)
    for core in range(N_CORES):
        out[core // 4] += res.results[core]["y"]
    out += b_proj[None, None, :]
    if trace:
        kernel.last_result = res
    return out


# revision 53
# speedup vs baseline: 1.0555x; 1.0555x over previous
# Trainium2 Bass kernel for a causal multi-head attention block.
#
# Reference computation (fp32):
#   qkv = x @ w_attn + b_attn ; split into q,k,v heads (N=16, H=64)
#   scores = q @ k^T / sqrt(H), causal mask, softmax over keys
#   out = (weights @ v) reshaped, then out @ w_proj + b_proj
#
# Sharding: 8 cores = 2 batches x 4 head-groups (4 heads each).
#   - batch data-parallel, heads tensor-parallel (c_attn columns / c_proj rows)
#   - each core emits a partial [T, D] projection output; host sums the 4
#     head-group partials per batch and adds b_proj (the gather step).
#
# On-device layout trick: scores are computed TRANSPOSED (S^T[s,t]) so that
# exp(S^T) tiles are directly usable as the stationary operand of the
# weights@V matmul (contraction over s = partition dim), eliminating all
# softmax-weight transposes.  Row sums come free via a ones-column in V.

import math

import numpy as np

B, T, D = 2, 2048, 1024
NHEAD, H = 16, 64
HPC = 4            # heads per core
CD = HPC * H       # 256 head-dim columns per core
N_CORES = 8
P = 128            # partitions
TT = T // P        # 16 t-tiles of 128
TB = T // 512      # 4 t-blocks of 512
KD = D // P        # 8 contraction tiles over D

_CACHE = {}


def _build_module(mm_dt_name: str):
    import contextlib

    import concourse.bass as bass  # noqa: F401
    import concourse.mybir as mybir
    import concourse.tile as tile
    from concourse import bacc

    f32 = mybir.dt.float32
    mdt = getattr(mybir.dt, mm_dt_name)
    # dtype for the softmax-weight @ V matmul operands: bf16 runs at
    # 1 cycle/row for any free-dim (fp32r pays 4x below N=256) and gets
    # fast weight loads on HW.  The row-sum is computed from the same
    # bf16 weights, so normalization cancels most of the rounding error.
    avdt = mybir.dt.bfloat16 if mm_dt_name == "float32r" else mdt

    nc = bacc.Bacc("TRN2", target_bir_lowering=False, debug=False)

    x_d = nc.dram_tensor("x", [T, D], mdt, kind="ExternalInput").ap()
    wqk_d = nc.dram_tensor("wqk", [D, 2 * CD], mdt, kind="ExternalInput").ap()
    bqk_d = nc.dram_tensor("bqk", [1, 2 * CD], mdt, kind="ExternalInput").ap()
    wv_d = nc.dram_tensor("wv", [D, CD], mdt, kind="ExternalInput").ap()
    bv_d = nc.dram_tensor("bv", [1, CD], mdt, kind="ExternalInput").ap()
    wp_d = nc.dram_tensor("wp", [CD, D], mdt, kind="ExternalInput").ap()
    ident_d = nc.dram_tensor("ident", [P, P], mdt, kind="ExternalInput").ap()
    mask_d = nc.dram_tensor("mask", [P, P], avdt, kind="ExternalInput").ap()
    ones_d = nc.dram_tensor("ones", [1, 512], mdt, kind="ExternalInput").ap()
    onescol_d = nc.dram_tensor("onescol", [P, 2 * HPC], avdt, kind="ExternalInput").ap()
    y_d = nc.dram_tensor("y", [T, D], f32, kind="ExternalOutput").ap()

    with tile.TileContext(nc) as tc, contextlib.ExitStack() as ctx:
        const_p = ctx.enter_context(tc.tile_pool(name="const", bufs=1))
        w_p = ctx.enter_context(tc.tile_pool(name="weights", bufs=1))
        x_p = ctx.enter_context(tc.tile_pool(name="xin", bufs=8))
        xt_p = ctx.enter_context(tc.tile_pool(name="xt", bufs=2))
        qkt_p = ctx.enter_context(tc.tile_pool(name="qkt", bufs=1))
        v_p = ctx.enter_context(tc.tile_pool(name="vbuf", bufs=1))
        e_p = ctx.enter_context(tc.tile_pool(name="epool", bufs=16))
        attn_p = ctx.enter_context(tc.tile_pool(name="attn", bufs=1))
        small_p = ctx.enter_context(tc.tile_pool(name="small", bufs=8))
        # single PSUM pool, 8 banks total:
        #   wps   [128,512]x2  (phase1 transposes/qkT/V + proj)      2 banks
        #   sp    [128,1024]x2 (scores)                              4 banks
        #   accp* [128,264]x2  (AV accumulators, 4 groups per bank)  2 banks
        psp = ctx.enter_context(tc.tile_pool(name="psp", bufs=2, space="PSUM"))

        # ---- loads, ordered by when phase 1 needs them, spread across
        # the three DMA-capable engines' queues ----
        x_dma_engines = [nc.sync, nc.scalar]
        rr = [0]

        def load(tile_ap, dram_ap):
            x_dma_engines[rr[0] % 2].dma_start(tile_ap, dram_ap)
            rr[0] += 1

        ident = const_p.tile([P, P], mdt, name="ident_sb")
        nc.sync.dma_start(ident, ident_d)

        x_sb = {}
        for g in range(4):
            xt_in = x_p.tile([P, D], mdt, name="x_sb", tag="x_sb")
            # keep the phase-gating first loads on the fast HWDGE queues
            # (SWDGE dispatch on gpsimd adds ~microseconds of latency)
            (nc.sync if g % 2 == 0 else nc.scalar).dma_start(
                xt_in, x_d[P * g : P * (g + 1), :]
            )
            x_sb[g] = xt_in

        wqk_sb = []
        for k in range(KD):
            t = w_p.tile([P, 2 * CD], mdt, name=f"wqk{k}", tag=f"wqk{k}")
            nc.sync.dma_start(t, wqk_d[P * k : P * (k + 1), :])
            wqk_sb.append(t)
        bqk = const_p.tile([1, 2 * CD], mdt, name="bqk_sb")
        nc.sync.dma_start(bqk, bqk_d)
        ones = const_p.tile([1, 512], mdt, name="ones_sb")
        nc.sync.dma_start(ones, ones_d)
        wv_sb = []
        for k in range(KD):
            t = w_p.tile([P, CD], mdt, name=f"wv{k}", tag=f"wv{k}")
            nc.sync.dma_start(t, wv_d[P * k : P * (k + 1), :])
            wv_sb.append(t)
        bv = const_p.tile([1, CD], mdt, name="bv_sb")
        nc.sync.dma_start(bv, bv_d)
        onescol = const_p.tile([P, 2 * HPC], avdt, name="onescol_sb")
        nc.sync.dma_start(onescol, onescol_d)
        mask = const_p.tile([P, P], avdt, name="mask_sb")
        nc.sync.dma_start(mask, mask_d)
        for g in range(4, 8):
            xt_in = x_p.tile([P, D], mdt, name="x_sb", tag="x_sb")
            load(xt_in, x_d[P * g : P * (g + 1), :])
            x_sb[g] = xt_in
        wp_sb = []
        for c in range(CD // P):
            t = w_p.tile([P, D], mdt, name=f"wp{c}", tag=f"wp{c}")
            nc.sync.dma_start(t, wp_d[P * c : P * (c + 1), :])
            wp_sb.append(t)

        # persistent activation buffers
        qkt_sb = {}
        for m in range(4):
            for j in range(TB):
                qkt_sb[(m, j)] = qkt_p.tile(
                    [P, 512], mdt, name=f"qkt{m}_{j}", tag=f"qkt{m}_{j}"
                )
        v_sb = []
        for i in range(TT):
            v_sb.append(
                v_p.tile([P, HPC * (H + 2)], avdt, name=f"v{i}", tag=f"v{i}")
            )
        attn_t = [
            attn_p.tile([P, 4 * CD], mdt, name=f"attn{c}", tag=f"attn{c}")
            for c in range(TB)
        ]
        G = H + 2

        def phase1_block(j):
            """x^T transposes + qk^T + V for 512-wide t-block j."""
            for ti in range(4):
                g = 4 * j + ti
                if g in x_sb:
                    continue
                xt_in = x_p.tile([P, D], mdt, name="x_sb", tag="x_sb")
                load(xt_in, x_d[P * g : P * (g + 1), :])
                x_sb[g] = xt_in
            xt_blk = []
            for k in range(KD):
                pt = psp.tile([P, 512], mdt, name="xtp", tag="wps")
                for ti in range(4):
                    nc.tensor.transpose(
                        pt[:, P * ti : P * (ti + 1)],
                        x_sb[4 * j + ti][:, P * k : P * (k + 1)],
                        ident,
                    )
                xt = xt_p.tile([P, 512], mdt, name=f"xt{k}", tag=f"xt{k}")
                nc.vector.tensor_copy(xt, pt)
                xt_blk.append(xt)

            for m in range(4):
                ps = psp.tile([P, 512], f32, name="qkp", tag="wps")
                nc.tensor.matmul(
                    ps,
                    bqk[0:1, P * m : P * (m + 1)],
                    ones[0:1, 0:512],
                    start=True,
                    stop=False,
                )
                for k in range(KD):
                    nc.tensor.matmul(
                        ps,
                        wqk_sb[k][:, P * m : P * (m + 1)],
                        xt_blk[k],
                        start=False,
                        stop=(k == KD - 1),
                    )
                nc.scalar.copy(qkt_sb[(m, j)], ps)

            for ti in range(4):
                g = 4 * j + ti
                ps = psp.tile([P, CD], f32, name="vp", tag="wps")
                nc.tensor.matmul(
                    ps,
                    ones[0:1, 0:P],
                    bv[0:1, :],
                    start=True,
                    stop=False,
                )
                for k in range(KD):
                    nc.tensor.matmul(
                        ps,
                        xt_blk[k][:, P * ti : P * (ti + 1)],
                        wv_sb[k],
                        start=False,
                        stop=(k == KD - 1),
                    )
                vg = v_sb[g].rearrange("p (g c) -> p g c", g=HPC)
                nc.scalar.copy(
                    vg[:, :, 0:H], ps.rearrange("p (g c) -> p g c", g=HPC)
                )
                nc.gpsimd.tensor_copy(
                    vg[:, :, H : H + 2],
                    onescol.rearrange("p (g c) -> p g c", c=2),
                )

        def attention(tb):
            """S^T -> exp -> AV for 512-wide t-block tb, heads processed in
            pairs (partition bases 0 and 64) so the two K=64 score matmuls
            occupy disjoint PE row groups and run concurrently; one psum
            tile holds both heads' scores so a single exp covers both."""
            for hp in range(2):
                h0, h1 = 2 * hp, 2 * hp + 1
                mq, mk = hp, 2 + hp
                acc_t = [
                    psp.tile([P, 4 * 66], f32, name="accp", tag=f"accp{a}",
                             bufs=1)
                    for a in range(2)
                ]
                n_s = 4 * tb + 4  # s-tiles 0 .. 4*tb+3
                for i in range(n_s):
                    first = max(0, i - 4 * tb)  # first valid jj in block
                    sps = psp.tile([P, 1024], f32, name="sp", tag="sp",
                                   bufs=2)
                    # trim fully-masked leading columns when it helps:
                    # fp32r matmuls below N=256 run at 1/4 rate, so only
                    # slice when the remaining width stays >= 256.
                    c0 = P * first if 512 - P * first >= 256 else 0
                    for hh, pb in ((0, 0), (1, 64)):
                        nc.tensor.matmul(
                            sps[:, 512 * hh + c0 : 512 * hh + 512],
                            qkt_sb[(mk, i // 4)][
                                pb : pb + H, P * (i % 4) : P * (i % 4 + 1)
                            ],
                            qkt_sb[(mq, tb)][pb : pb + H, c0:512],
                            start=True,
                            stop=True,
                        )
                    et = e_p.tile([P, 1024], avdt, name="et", tag="et")
                    if first:
                        w = 512 - P * first
                        nc.scalar.activation(
                            et.rearrange("p (g c) -> p g c", g=2)[
                                :, :, P * first : 512
                            ],
                            sps.rearrange("p (g c) -> p g c", g=2)[
                                :, :, P * first : 512
                            ],
                            mybir.ActivationFunctionType.Exp,
                            scale=1.0 / math.sqrt(H),
                        )
                    else:
                        nc.scalar.activation(
                            et,
                            sps,
                            mybir.ActivationFunctionType.Exp,
                            scale=1.0 / math.sqrt(H),
                        )
                    if 0 <= i - 4 * tb <= 3:
                        jj = i - 4 * tb
                        for hh in range(2):
                            nc.vector.tensor_mul(
                                et[:, 512 * hh + P * jj : 512 * hh + P * (jj + 1)],
                                et[:, 512 * hh + P * jj : 512 * hh + P * (jj + 1)],
                                mask,
                            )
                    for jj in range(first, 4):
                        jglob = 4 * tb + jj
                        for hh in range(2):
                            # start=True clears has_written for the WHOLE
                            # psum bank: only the first group per bank
                            # issues it.
                            nc.tensor.matmul(
                                acc_t[hh][:, 66 * jj : 66 * jj + 66],
                                et[:, 512 * hh + P * jj : 512 * hh + P * (jj + 1)],
                                v_sb[i][:, G * (h0 + hh) : G * (h0 + hh) + 66],
                                start=(i == 0 and jj == 0),
                                stop=(i == jglob),
                                skip_group_check=True,
                            )
                for jj in range(4):
                    for hh in range(2):
                        h = h0 + hh
                        s0 = 66 * jj
                        rec = small_p.tile([P, 1], f32, name="rec", tag="rec")
                        nc.vector.reciprocal(
                            rec, acc_t[hh][:, s0 + H : s0 + H + 1]
                        )
                        nc.vector.tensor_scalar_mul(
                            attn_t[tb][:, CD * jj + H * h : CD * jj + H * (h + 1)],
                            acc_t[hh][:, s0 : s0 + H],
                            rec,
                        )

        def projection(jb):
            """attn^T transposes + y = attn @ wp for 512-wide t-block jb."""
            attnT = {}
            for c in range(CD // P):
                pt = psp.tile([P, 512], mdt, name="atp", tag="wps")
                for ti in range(4):
                    nc.tensor.transpose(
                        pt[:, P * ti : P * (ti + 1)],
                        attn_t[jb][:, CD * ti + P * c : CD * ti + P * (c + 1)],
                        ident,
                    )
                at = e_p.tile([P, 512], mdt, name="at", tag="at", bufs=4)
                if jb == 3 and c == 0:
                    nc.scalar.copy(at, pt)
                else:
                    nc.vector.tensor_copy(at, pt)
                attnT[c] = at

            for jl in range(4):
                jt = 4 * jb + jl
                for n in range(2):
                    # block 3's projection runs after all attention: the
                    # score psum slots are free then
                    ps = psp.tile([P, 512], f32, name="yp",
                                  tag=("sp" if jb == 3 else "wps"))
                    for c in range(CD // P):
                        nc.tensor.matmul(
                            ps,
                            attnT[c][:, P * jl : P * (jl + 1)],
                            wp_sb[c][:, 512 * n : 512 * (n + 1)],
                            start=(c == 0),
                            stop=(c == CD // P - 1),
                        )
                    ysb = small_p.tile([P, 512], f32, name="ysb", tag="ysb",
                                       bufs=4)
                    if jb == 3 and (jl + n) % 2 == 0:
                        nc.scalar.copy(ysb, ps)
                    else:
                        nc.vector.tensor_copy(ysb, ps)
                    (nc.sync if (jb < 3 or n == 0) else nc.scalar).dma_start(
                        y_d[P * jt : P * (jt + 1), 512 * n : 512 * (n + 1)],
                        ysb,
                    )

        # emission order chosen so chunk-0 attention (ACT-bound) can overlap
        # the second half of phase 1 (PE-bound), and each chunk's projection
        # overlaps the next chunk's attention.
        phase1_block(0)
        attention(0)
        phase1_block(1)
        attention(1)
        phase1_block(2)
        attention(2)
        phase1_block(3)
        attention(3)
        projection(0)
        projection(1)
        projection(2)
        projection(3)

    nc.compile()
    return nc


def _get_module(mm_dt_name: str):
    if mm_dt_name not in _CACHE:
        _CACHE[mm_dt_name] = _build_module(mm_dt_name)
    return _CACHE[mm_dt_name]


def kernel(x, w_attn, b_attn, w_proj, b_proj, mm_dt_name: str = "float32r",
           trace: bool = False):
    from concourse.bass_utils import run_bass_kernel_spmd

    x = np.asarray(x, dtype=np.float32)
    w_attn = np.asarray(w_attn, dtype=np.float32)
    b_attn = np.asarray(b_attn, dtype=np.float32)
    w_proj = np.asarray(w_proj, dtype=np.float32)
    b_proj = np.asarray(b_proj, dtype=np.float32)

    nc = _get_module(mm_dt_name)

    import ml_dtypes

    avnp = np.dtype(ml_dtypes.bfloat16) if mm_dt_name == "float32r" else np.float32
    ident = np.eye(P, dtype=np.float32)
    mask = np.triu(np.ones((P, P), dtype=avnp))
    ones = np.ones((1, 512), dtype=np.float32)

    in_maps = []
    for core in range(N_CORES):
        b = core // 4
        g = core % 4
        c0 = CD * g
        wq = w_attn[:, c0 : c0 + CD]
        wk = w_attn[:, D + c0 : D + c0 + CD]
        wv = w_attn[:, 2 * D + c0 : 2 * D + c0 + CD]
        bq = b_attn[c0 : c0 + CD]
        bk = b_attn[D + c0 : D + c0 + CD]
        bvv = b_attn[2 * D + c0 : 2 * D + c0 + CD]
        in_maps.append(
            {
                "x": np.ascontiguousarray(x[b]),
                "wqk": np.ascontiguousarray(np.concatenate([wq, wk], axis=1)),
                "bqk": np.concatenate([bq, bk])[None, :].copy(),
                "wv": np.ascontiguousarray(wv),
                "bv": bvv[None, :].copy(),
                "wp": np.ascontiguousarray(w_proj[c0 : c0 + CD, :]),
                "ident": ident,
                "mask": mask,
                "ones": ones,
                "onescol": np.tile(np.array([1.0, 0.0], avnp), (P, HPC)),
            }
        )

    res = run_bass_kernel_spmd(
        nc, in_maps, core_ids=list(range(N_CORES)), trace=trace
    )

    out = np.zeros((B, T, D), dtype=np.float32)
    for core in range(N_CORES):
        out[core // 4] += res.results[core]["y"]
    out += b_proj[None, None, :]
    if trace:
        kernel.last_result = res
    return out


# revision 64
# speedup vs baseline: 1.0581x; 1.0024x over previous
# Trainium2 Bass kernel for a causal multi-head attention block.
#
# Reference computation (fp32):
#   qkv = x @ w_attn + b_attn ; split into q,k,v heads (N=16, H=64)
#   scores = q @ k^T / sqrt(H), causal mask, softmax over keys
#   out = (weights @ v) reshaped, then out @ w_proj + b_proj
#
# Sharding: 8 cores = 2 batches x 4 head-groups (4 heads each).
#   - batch data-parallel, heads tensor-parallel (c_attn columns / c_proj rows)
#   - each core emits a partial [T, D] projection output; host sums the 4
#     head-group partials per batch and adds b_proj (the gather step).
#
# On-device layout trick: scores are computed TRANSPOSED (S^T[s,t]) so that
# exp(S^T) tiles are directly usable as the stationary operand of the
# weights@V matmul (contraction over s = partition dim), eliminating all
# softmax-weight transposes.  Row sums come free via a ones-column in V.

import math

import numpy as np

B, T, D = 2, 2048, 1024
NHEAD, H = 16, 64
HPC = 4            # heads per core
CD = HPC * H       # 256 head-dim columns per core
N_CORES = 8
P = 128            # partitions
TT = T // P        # 16 t-tiles of 128
TB = T // 512      # 4 t-blocks of 512
KD = D // P        # 8 contraction tiles over D

_CACHE = {}


def _build_module(mm_dt_name: str):
    import contextlib

    import concourse.bass as bass  # noqa: F401
    import concourse.mybir as mybir
    import concourse.tile as tile
    from concourse import bacc

    f32 = mybir.dt.float32
    mdt = getattr(mybir.dt, mm_dt_name)
    # dtype for the softmax-weight @ V matmul operands: bf16 runs at
    # 1 cycle/row for any free-dim (fp32r pays 4x below N=256) and gets
    # fast weight loads on HW.  The row-sum is computed from the same
    # bf16 weights, so normalization cancels most of the rounding error.
    avdt = mybir.dt.bfloat16 if mm_dt_name == "float32r" else mdt

    nc = bacc.Bacc("TRN2", target_bir_lowering=False, debug=False)

    x_d = nc.dram_tensor("x", [T, D], mdt, kind="ExternalInput").ap()
    wqk_d = nc.dram_tensor("wqk", [D, 2 * CD], mdt, kind="ExternalInput").ap()
    bqk_d = nc.dram_tensor("bqk", [1, 2 * CD], mdt, kind="ExternalInput").ap()
    wv_d = nc.dram_tensor("wv", [D, CD], mdt, kind="ExternalInput").ap()
    bv_d = nc.dram_tensor("bv", [1, CD], mdt, kind="ExternalInput").ap()
    wp_d = nc.dram_tensor("wp", [CD, D], mdt, kind="ExternalInput").ap()
    ident_d = nc.dram_tensor("ident", [P, P], mdt, kind="ExternalInput").ap()
    mask_d = nc.dram_tensor("mask", [P, P], avdt, kind="ExternalInput").ap()
    ones_d = nc.dram_tensor("ones", [1, 512], mdt, kind="ExternalInput").ap()
    onescol_d = nc.dram_tensor("onescol", [P, 2 * HPC], avdt, kind="ExternalInput").ap()
    y_d = nc.dram_tensor("y", [T, D], f32, kind="ExternalOutput").ap()

    with tile.TileContext(nc) as tc, contextlib.ExitStack() as ctx:
        const_p = ctx.enter_context(tc.tile_pool(name="const", bufs=1))
        w_p = ctx.enter_context(tc.tile_pool(name="weights", bufs=1))
        x_p = ctx.enter_context(tc.tile_pool(name="xin", bufs=8))
        xt_p = ctx.enter_context(tc.tile_pool(name="xt", bufs=2))
        qkt_p = ctx.enter_context(tc.tile_pool(name="qkt", bufs=1))
        v_p = ctx.enter_context(tc.tile_pool(name="vbuf", bufs=1))
        e_p = ctx.enter_context(tc.tile_pool(name="epool", bufs=16))
        attn_p = ctx.enter_context(tc.tile_pool(name="attn", bufs=1))
        small_p = ctx.enter_context(tc.tile_pool(name="small", bufs=8))
        # single PSUM pool, 8 banks total:
        #   wps   [128,512]x2  (phase1 transposes/qkT/V + proj)      2 banks
        #   sp    [128,1024]x2 (scores)                              4 banks
        #   accp* [128,264]x2  (AV accumulators, 4 groups per bank)  2 banks
        psp = ctx.enter_context(tc.tile_pool(name="psp", bufs=2, space="PSUM"))

        # ---- loads, ordered by when phase 1 needs them, spread across
        # the three DMA-capable engines' queues ----
        x_dma_engines = [nc.sync, nc.scalar]
        rr = [0]

        def load(tile_ap, dram_ap):
            x_dma_engines[rr[0] % 2].dma_start(tile_ap, dram_ap)
            rr[0] += 1

        ident = const_p.tile([P, P], mdt, name="ident_sb")
        nc.sync.dma_start(ident, ident_d)

        x_sb = {}
        for g in range(4):
            xt_in = x_p.tile([P, D], mdt, name="x_sb", tag="x_sb")
            # keep the phase-gating first loads on the fast HWDGE queues
            # (SWDGE dispatch on gpsimd adds ~microseconds of latency)
            (nc.sync if g % 2 == 0 else nc.scalar).dma_start(
                xt_in, x_d[P * g : P * (g + 1), :]
            )
            x_sb[g] = xt_in

        bqk = const_p.tile([1, 2 * CD], mdt, name="bqk_sb")
        nc.sync.dma_start(bqk, bqk_d)
        ones = const_p.tile([1, 512], mdt, name="ones_sb")
        nc.sync.dma_start(ones, ones_d)
        wqk_sb = []
        for k in range(KD):
            t = w_p.tile([P, 2 * CD], mdt, name=f"wqk{k}", tag=f"wqk{k}")
            nc.sync.dma_start(t, wqk_d[P * k : P * (k + 1), :])
            wqk_sb.append(t)
        wv_sb = []
        for k in range(KD):
            t = w_p.tile([P, CD], mdt, name=f"wv{k}", tag=f"wv{k}")
            nc.sync.dma_start(t, wv_d[P * k : P * (k + 1), :])
            wv_sb.append(t)
        bv = const_p.tile([1, CD], mdt, name="bv_sb")
        nc.sync.dma_start(bv, bv_d)
        onescol = const_p.tile([P, 2 * HPC], avdt, name="onescol_sb")
        nc.sync.dma_start(onescol, onescol_d)
        mask = const_p.tile([P, P], avdt, name="mask_sb")
        nc.sync.dma_start(mask, mask_d)
        for g in range(4, 8):
            xt_in = x_p.tile([P, D], mdt, name="x_sb", tag="x_sb")
            load(xt_in, x_d[P * g : P * (g + 1), :])
            x_sb[g] = xt_in
        wp_sb = []
        for c in range(CD // P):
            t = w_p.tile([P, D], mdt, name=f"wp{c}", tag=f"wp{c}")
            nc.sync.dma_start(t, wp_d[P * c : P * (c + 1), :])
            wp_sb.append(t)

        # persistent activation buffers
        qkt_sb = {}
        for m in range(4):
            for j in range(TB):
                qkt_sb[(m, j)] = qkt_p.tile(
                    [P, 512], mdt, name=f"qkt{m}_{j}", tag=f"qkt{m}_{j}"
                )
        v_sb = []
        for i in range(TT):
            v_sb.append(
                v_p.tile([P, HPC * (H + 2)], avdt, name=f"v{i}", tag=f"v{i}")
            )
        # one tile per (t-block, head-pair): the projection's c-th
        # transpose batch depends only on head-pair c's normalizes
        attn_t = {
            (tb, c): attn_p.tile([P, 4 * P], mdt, name=f"attn{tb}_{c}",
                                 tag=f"attn{tb}_{c}")
            for tb in range(TB)
            for c in range(2)
        }
        G = H + 2

        def phase1_block(j):
            """x^T transposes + qk^T + V for 512-wide t-block j."""
            for ti in range(4):
                g = 4 * j + ti
                if g in x_sb:
                    continue
                xt_in = x_p.tile([P, D], mdt, name="x_sb", tag="x_sb")
                load(xt_in, x_d[P * g : P * (g + 1), :])
                x_sb[g] = xt_in
            xt_blk = []
            for k in range(KD):
                pt = psp.tile([P, 512], mdt, name="xtp", tag="wps")
                for ti in range(4):
                    nc.tensor.transpose(
                        pt[:, P * ti : P * (ti + 1)],
                        x_sb[4 * j + ti][:, P * k : P * (k + 1)],
                        ident,
                    )
                xt = xt_p.tile([P, 512], mdt, name=f"xt{k}", tag=f"xt{k}")
                nc.vector.tensor_copy(xt, pt)
                xt_blk.append(xt)

            for m in (0, 2, 1, 3):  # head-pair 0 needs m0+m2: finish first
                ps = psp.tile([P, 512], f32, name="qkp", tag="wps")
                nc.tensor.matmul(
                    ps,
                    bqk[0:1, P * m : P * (m + 1)],
                    ones[0:1, 0:512],
                    start=True,
                    stop=False,
                )
                for k in range(KD):
                    nc.tensor.matmul(
                        ps,
                        wqk_sb[k][:, P * m : P * (m + 1)],
                        xt_blk[k],
                        start=False,
                        stop=(k == KD - 1),
                    )
                nc.scalar.copy(qkt_sb[(m, j)], ps)

            for ti in range(4):
                g = 4 * j + ti
                ps = psp.tile([P, CD], f32, name="vp", tag="wps")
                nc.tensor.matmul(
                    ps,
                    ones[0:1, 0:P],
                    bv[0:1, :],
                    start=True,
                    stop=False,
                )
                for k in range(KD):
                    nc.tensor.matmul(
                        ps,
                        xt_blk[k][:, P * ti : P * (ti + 1)],
                        wv_sb[k],
                        start=False,
                        stop=(k == KD - 1),
                    )
                vg = v_sb[g].rearrange("p (g c) -> p g c", g=HPC)
                nc.scalar.copy(
                    vg[:, :, 0:H], ps.rearrange("p (g c) -> p g c", g=HPC)
                )
                nc.gpsimd.tensor_copy(
                    vg[:, :, H : H + 2],
                    onescol.rearrange("p (g c) -> p g c", c=2),
                )

        def attention(tb):
            """S^T -> exp -> AV for 512-wide t-block tb, heads processed in
            pairs (partition bases 0 and 64) so the two K=64 score matmuls
            occupy disjoint PE row groups and run concurrently; one psum
            tile holds both heads' scores so a single exp covers both."""
            for hp in range(2):
                h0 = 2 * hp
                mq, mk = hp, 2 + hp
                acc_t = [
                    psp.tile([P, 4 * 66], f32, name="accp", tag=f"accp{a}",
                             bufs=1)
                    for a in range(2)
                ]
                n_s = 4 * tb + 4  # s-tiles 0 .. 4*tb+3
                for i in range(n_s):
                    first = max(0, i - 4 * tb)  # first valid jj in block
                    sps = psp.tile([P, 1024], f32, name="sp", tag="sp",
                                   bufs=2)
                    # trim fully-masked leading columns when it helps:
                    # fp32r matmuls below N=256 run at 1/4 rate, so only
                    # slice when the remaining width stays >= 256.
                    c0 = P * first if 512 - P * first >= 256 else 0
                    for hh, pb in ((0, 0), (1, 64)):
                        nc.tensor.matmul(
                            sps[:, 512 * hh + c0 : 512 * hh + 512],
                            qkt_sb[(mk, i // 4)][
                                pb : pb + H, P * (i % 4) : P * (i % 4 + 1)
                            ],
                            qkt_sb[(mq, tb)][pb : pb + H, c0:512],
                            start=True,
                            stop=True,
                        )
                    et = e_p.tile([P, 1024], avdt, name="et", tag="et")
                    if first:
                        nc.scalar.activation(
                            et.rearrange("p (g c) -> p g c", g=2)[
                                :, :, P * first : 512
                            ],
                            sps.rearrange("p (g c) -> p g c", g=2)[
                                :, :, P * first : 512
                            ],
                            mybir.ActivationFunctionType.Exp,
                            scale=1.0 / math.sqrt(H),
                        )
                    else:
                        nc.scalar.activation(
                            et,
                            sps,
                            mybir.ActivationFunctionType.Exp,
                            scale=1.0 / math.sqrt(H),
                        )
                    dj = i - 4 * tb  # diagonal jj of this s-tile, if any
                    etd = None
                    if 0 <= dj <= 3:
                        # masked diagonal sub-tiles go to a separate tile so
                        # the non-diagonal AV matmuls don't serialize behind
                        # the mask write (tile-granular dependency tracking)
                        etd = e_p.tile([P, 2 * P], avdt, name="etd", tag="etd",
                                       bufs=2)
                        for hh in range(2):
                            nc.vector.tensor_mul(
                                etd[:, P * hh : P * (hh + 1)],
                                et[:, 512 * hh + P * dj : 512 * hh + P * (dj + 1)],
                                mask,
                            )
                    for jj in range(first, 4):
                        jglob = 4 * tb + jj
                        for hh in range(2):
                            if jj == dj:
                                lhs_e = etd[:, P * hh : P * (hh + 1)]
                            else:
                                lhs_e = et[
                                    :, 512 * hh + P * jj : 512 * hh + P * (jj + 1)
                                ]
                            # start=True clears has_written for the WHOLE
                            # psum bank: only the first group per bank
                            # issues it.
                            nc.tensor.matmul(
                                acc_t[hh][:, 66 * jj : 66 * jj + 66],
                                lhs_e,
                                v_sb[i][:, G * (h0 + hh) : G * (h0 + hh) + 66],
                                start=(i == 0 and jj == 0),
                                stop=(i == jglob),
                                skip_group_check=True,
                            )
                for jj in range(4):
                    for hh in range(2):
                        h = h0 + hh
                        s0 = 66 * jj
                        rec = small_p.tile([P, 1], f32, name="rec", tag="rec")
                        nc.vector.reciprocal(
                            rec, acc_t[hh][:, s0 + H : s0 + H + 1]
                        )
                        nc.vector.tensor_scalar_mul(
                            attn_t[(tb, hp)][
                                :, P * jj + H * hh : P * jj + H * (hh + 1)
                            ],
                            acc_t[hh][:, s0 : s0 + H],
                            rec,
                        )

        def projection(jb):
            """attn^T transposes + y = attn @ wp for 512-wide t-block jb."""
            attnT = {}
            for c in range(CD // P):
                pt = psp.tile([P, 512], mdt, name="atp", tag="wps")
                for ti in range(4):
                    nc.tensor.transpose(
                        pt[:, P * ti : P * (ti + 1)],
                        attn_t[(jb, c)][:, P * ti : P * (ti + 1)],
                        ident,
                    )
                at = e_p.tile([P, 512], mdt, name="at", tag="at", bufs=4)
                if jb == 3 and c == 0:
                    nc.scalar.copy(at, pt)
                else:
                    nc.vector.tensor_copy(at, pt)
                attnT[c] = at

            for jl in range(4):
                jt = 4 * jb + jl
                for n in range(2):
                    # block 3's projection runs after all attention: the
                    # score psum slots are free then
                    ps = psp.tile([P, 512], f32, name="yp",
                                  tag=("sp" if jb == 3 else "wps"))
                    for c in range(CD // P):
                        nc.tensor.matmul(
                            ps,
                            attnT[c][:, P * jl : P * (jl + 1)],
                            wp_sb[c][:, 512 * n : 512 * (n + 1)],
                            start=(c == 0),
                            stop=(c == CD // P - 1),
                        )
                    ysb = small_p.tile([P, 512], f32, name="ysb", tag="ysb",
                                       bufs=4)
                    if jb == 3 and (jl + n) % 2 == 0:
                        nc.scalar.copy(ysb, ps)
                    else:
                        nc.vector.tensor_copy(ysb, ps)
                    (nc.sync if (jb < 3 or n == 0) else nc.scalar).dma_start(
                        y_d[P * jt : P * (jt + 1), 512 * n : 512 * (n + 1)],
                        ysb,
                    )

        # emission order chosen so chunk-0 attention (ACT-bound) can overlap
        # the second half of phase 1 (PE-bound), and each chunk's projection
        # overlaps the next chunk's attention.
        phase1_block(0)
        attention(0)
        phase1_block(1)
        attention(1)
        phase1_block(2)
        attention(2)
        phase1_block(3)
        attention(3)
        projection(0)
        projection(1)
        projection(2)
        projection(3)

    nc.compile()
    return nc


def _get_module(mm_dt_name: str):
    if mm_dt_name not in _CACHE:
        _CACHE[mm_dt_name] = _build_module(mm_dt_name)
    return _CACHE[mm_dt_name]


def kernel(x, w_attn, b_attn, w_proj, b_proj, mm_dt_name: str = "float32r",
           trace: bool = False):
    from concourse.bass_utils import run_bass_kernel_spmd

    x = np.asarray(x, dtype=np.float32)
    w_attn = np.asarray(w_attn, dtype=np.float32)
    b_attn = np.asarray(b_attn, dtype=np.float32)
    w_proj = np.asarray(w_proj, dtype=np.float32)
    b_proj = np.asarray(b_proj, dtype=np.float32)

    nc = _get_module(mm_dt_name)

    import ml_dtypes

    avnp = np.dtype(ml_dtypes.bfloat16) if mm_dt_name == "float32r" else np.float32
    ident = np.eye(P, dtype=np.float32)
    mask = np.triu(np.ones((P, P), dtype=avnp))
    ones = np.ones((1, 512), dtype=np.float32)

    in_maps = []
    for core in range(N_CORES):
        b = core // 4
        g = core % 4
        c0 = CD * g
        wq = w_attn[:, c0 : c0 + CD]
        wk = w_attn[:, D + c0 : D + c0 + CD]
        wv = w_attn[:, 2 * D + c0 : 2 * D + c0 + CD]
        bq = b_attn[c0 : c0 + CD]
        bk = b_attn[D + c0 : D + c0 + CD]
        bvv = b_attn[2 * D + c0 : 2 * D + c0 + CD]
        in_maps.append(
            {
                "x": np.ascontiguousarray(x[b]),
                "wqk": np.ascontiguousarray(np.concatenate([wq, wk], axis=1)),
                "bqk": np.concatenate([bq, bk])[None, :].copy(),
                "wv": np.ascontiguousarray(wv),
                "bv": bvv[None, :].copy(),
                "wp": np.ascontiguousarray(w_proj[c0 : c0 + CD, :]),
                "ident": ident,
                "mask": mask,
                "ones": ones,
                "onescol": np.tile(np.array([1.0, 0.0], avnp), (P, HPC)),
            }
        )

    res = run_bass_kernel_spmd(
        nc, in_maps, core_ids=list(range(N_CORES)), trace=trace
    )

    out = np.zeros((B, T, D), dtype=np.float32)
    for core in range(N_CORES):
        out[core // 4] += res.results[core]["y"]
    out += b_proj[None, None, :]
    if trace:
        kernel.last_result = res
    return out


# revision 67
# speedup vs baseline: 1.0588x; 1.0006x over previous
# Trainium2 Bass kernel for a causal multi-head attention block.
#
# Reference computation (fp32):
#   qkv = x @ w_attn + b_attn ; split into q,k,v heads (N=16, H=64)
#   scores = q @ k^T / sqrt(H), causal mask, softmax over keys
#   out = (weights @ v) reshaped, then out @ w_proj + b_proj
#
# Sharding: 8 cores = 2 batches x 4 head-groups (4 heads each).
#   - batch data-parallel, heads tensor-parallel (c_attn columns / c_proj rows)
#   - each core emits a partial [T, D] projection output; host sums the 4
#     head-group partials per batch and adds b_proj (the gather step).
#
# On-device layout trick: scores are computed TRANSPOSED (S^T[s,t]) so that
# exp(S^T) tiles are directly usable as the stationary operand of the
# weights@V matmul (contraction over s = partition dim), eliminating all
# softmax-weight transposes.  Row sums come free via a ones-column in V.

import math

import numpy as np

B, T, D = 2, 2048, 1024
NHEAD, H = 16, 64
HPC = 4            # heads per core
CD = HPC * H       # 256 head-dim columns per core
N_CORES = 8
P = 128            # partitions
TT = T // P        # 16 t-tiles of 128
TB = T // 512      # 4 t-blocks of 512
KD = D // P        # 8 contraction tiles over D

_CACHE = {}


def _build_module(mm_dt_name: str):
    import contextlib

    import concourse.bass as bass  # noqa: F401
    import concourse.mybir as mybir
    import concourse.tile as tile
    from concourse import bacc

    f32 = mybir.dt.float32
    mdt = getattr(mybir.dt, mm_dt_name)
    # dtype for the softmax-weight @ V matmul operands: bf16 runs at
    # 1 cycle/row for any free-dim (fp32r pays 4x below N=256) and gets
    # fast weight loads on HW.  The row-sum is computed from the same
    # bf16 weights, so normalization cancels most of the rounding error.
    avdt = mybir.dt.bfloat16 if mm_dt_name == "float32r" else mdt

    nc = bacc.Bacc("TRN2", target_bir_lowering=False, debug=False)

    x_d = nc.dram_tensor("x", [T, D], mdt, kind="ExternalInput").ap()
    wqk_d = nc.dram_tensor("wqk", [D, 2 * CD], mdt, kind="ExternalInput").ap()
    bqk_d = nc.dram_tensor("bqk", [1, 2 * CD], mdt, kind="ExternalInput").ap()
    wv_d = nc.dram_tensor("wv", [D, CD], mdt, kind="ExternalInput").ap()
    bv_d = nc.dram_tensor("bv", [1, CD], mdt, kind="ExternalInput").ap()
    wp_d = nc.dram_tensor("wp", [CD, D], mdt, kind="ExternalInput").ap()
    ident_d = nc.dram_tensor("ident", [P, P], mdt, kind="ExternalInput").ap()
    mask_d = nc.dram_tensor("mask", [P, P], avdt, kind="ExternalInput").ap()
    ones_d = nc.dram_tensor("ones", [1, 512], mdt, kind="ExternalInput").ap()
    onescol_d = nc.dram_tensor("onescol", [P, 2 * HPC], avdt, kind="ExternalInput").ap()
    y_d = nc.dram_tensor("y", [T, D], f32, kind="ExternalOutput").ap()

    with tile.TileContext(nc) as tc, contextlib.ExitStack() as ctx:
        const_p = ctx.enter_context(tc.tile_pool(name="const", bufs=1))
        w_p = ctx.enter_context(tc.tile_pool(name="weights", bufs=1))
        x_p = ctx.enter_context(tc.tile_pool(name="xin", bufs=8))
        xt_p = ctx.enter_context(tc.tile_pool(name="xt", bufs=2))
        qkt_p = ctx.enter_context(tc.tile_pool(name="qkt", bufs=1))
        v_p = ctx.enter_context(tc.tile_pool(name="vbuf", bufs=1))
        e_p = ctx.enter_context(tc.tile_pool(name="epool", bufs=16))
        attn_p = ctx.enter_context(tc.tile_pool(name="attn", bufs=1))
        small_p = ctx.enter_context(tc.tile_pool(name="small", bufs=8))
        # single PSUM pool, 8 banks total:
        #   wps   [128,512]x2  (phase1 transposes/qkT/V + proj)      2 banks
        #   sp    [128,1024]x2 (scores)                              4 banks
        #   accp* [128,264]x2  (AV accumulators, 4 groups per bank)  2 banks
        psp = ctx.enter_context(tc.tile_pool(name="psp", bufs=2, space="PSUM"))

        # ---- loads, ordered by when phase 1 needs them, spread across
        # the three DMA-capable engines' queues ----
        x_dma_engines = [nc.sync, nc.scalar]
        rr = [0]

        def load(tile_ap, dram_ap):
            x_dma_engines[rr[0] % 2].dma_start(tile_ap, dram_ap)
            rr[0] += 1

        ident = const_p.tile([P, P], mdt, name="ident_sb")
        nc.sync.dma_start(ident, ident_d)

        x_sb = {}
        for g in range(4):
            xt_in = x_p.tile([P, D], mdt, name="x_sb", tag="x_sb")
            # keep the phase-gating first loads on the fast HWDGE queues
            # (SWDGE dispatch on gpsimd adds ~microseconds of latency)
            (nc.sync if g % 2 == 0 else nc.scalar).dma_start(
                xt_in, x_d[P * g : P * (g + 1), :]
            )
            x_sb[g] = xt_in

        bqk = const_p.tile([1, 2 * CD], mdt, name="bqk_sb")
        nc.sync.dma_start(bqk, bqk_d)
        ones = const_p.tile([1, 512], mdt, name="ones_sb")
        nc.sync.dma_start(ones, ones_d)
        wqk_sb = []
        for k in range(KD):
            t = w_p.tile([P, 2 * CD], mdt, name=f"wqk{k}", tag=f"wqk{k}")
            nc.sync.dma_start(t, wqk_d[P * k : P * (k + 1), :])
            wqk_sb.append(t)
        wv_sb = []
        for k in range(KD):
            t = w_p.tile([P, CD], mdt, name=f"wv{k}", tag=f"wv{k}")
            nc.sync.dma_start(t, wv_d[P * k : P * (k + 1), :])
            wv_sb.append(t)
        bv = const_p.tile([1, CD], mdt, name="bv_sb")
        nc.sync.dma_start(bv, bv_d)
        onescol = const_p.tile([P, 2 * HPC], avdt, name="onescol_sb")
        nc.sync.dma_start(onescol, onescol_d)
        mask = const_p.tile([P, P], avdt, name="mask_sb")
        nc.sync.dma_start(mask, mask_d)
        for g in range(4, 8):
            xt_in = x_p.tile([P, D], mdt, name="x_sb", tag="x_sb")
            load(xt_in, x_d[P * g : P * (g + 1), :])
            x_sb[g] = xt_in
        wp_sb = []
        for c in range(CD // P):
            t = w_p.tile([P, D], mdt, name=f"wp{c}", tag=f"wp{c}")
            nc.sync.dma_start(t, wp_d[P * c : P * (c + 1), :])
            wp_sb.append(t)

        # persistent activation buffers
        qkt_sb = {}
        for m in range(4):
            for j in range(TB):
                qkt_sb[(m, j)] = qkt_p.tile(
                    [P, 512], mdt, name=f"qkt{m}_{j}", tag=f"qkt{m}_{j}"
                )
        v_sb = []
        for i in range(TT):
            v_sb.append(
                v_p.tile([P, HPC * (H + 2)], avdt, name=f"v{i}", tag=f"v{i}")
            )
        # one tile per (t-block, head-pair): the projection's c-th
        # transpose batch depends only on head-pair c's normalizes
        attn_t = {
            (tb, c): attn_p.tile([P, 4 * P], mdt, name=f"attn{tb}_{c}",
                                 tag=f"attn{tb}_{c}")
            for tb in range(TB)
            for c in range(2)
        }
        G = H + 2

        def phase1_block(j):
            """x^T transposes + qk^T + V for 512-wide t-block j."""
            for ti in range(4):
                g = 4 * j + ti
                if g in x_sb:
                    continue
                xt_in = x_p.tile([P, D], mdt, name="x_sb", tag="x_sb")
                load(xt_in, x_d[P * g : P * (g + 1), :])
                x_sb[g] = xt_in
            xt_blk = []
            for k in range(KD):
                pt = psp.tile([P, 512], mdt, name="xtp", tag="wps")
                for ti in range(4):
                    nc.tensor.transpose(
                        pt[:, P * ti : P * (ti + 1)],
                        x_sb[4 * j + ti][:, P * k : P * (k + 1)],
                        ident,
                    )
                xt = xt_p.tile([P, 512], mdt, name=f"xt{k}", tag=f"xt{k}")
                nc.vector.tensor_copy(xt, pt)
                xt_blk.append(xt)

            for m in (0, 2, 1, 3):  # head-pair 0 needs m0+m2: finish first
                ps = psp.tile([P, 512], f32, name="qkp", tag="wps")
                nc.tensor.matmul(
                    ps,
                    bqk[0:1, P * m : P * (m + 1)],
                    ones[0:1, 0:512],
                    start=True,
                    stop=False,
                )
                for k in range(KD):
                    nc.tensor.matmul(
                        ps,
                        wqk_sb[k][:, P * m : P * (m + 1)],
                        xt_blk[k],
                        start=False,
                        stop=(k == KD - 1),
                    )
                nc.scalar.copy(qkt_sb[(m, j)], ps)

            for ti in range(4):
                g = 4 * j + ti
                ps = psp.tile([P, CD], f32, name="vp", tag="wps")
                nc.tensor.matmul(
                    ps,
                    ones[0:1, 0:P],
                    bv[0:1, :],
                    start=True,
                    stop=False,
                )
                for k in range(KD):
                    nc.tensor.matmul(
                        ps,
                        xt_blk[k][:, P * ti : P * (ti + 1)],
                        wv_sb[k],
                        start=False,
                        stop=(k == KD - 1),
                    )
                vg = v_sb[g].rearrange("p (g c) -> p g c", g=HPC)
                nc.scalar.copy(
                    vg[:, :, 0:H], ps.rearrange("p (g c) -> p g c", g=HPC)
                )
                nc.gpsimd.tensor_copy(
                    vg[:, :, H : H + 2],
                    onescol.rearrange("p (g c) -> p g c", c=2),
                )

        def attention(tb):
            """S^T -> exp -> AV for 512-wide t-block tb, heads processed in
            pairs (partition bases 0 and 64) so the two K=64 score matmuls
            occupy disjoint PE row groups and run concurrently; one psum
            tile holds both heads' scores so a single exp covers both."""
            for hp in range(2):
                h0 = 2 * hp
                mq, mk = hp, 2 + hp
                acc_t = [
                    psp.tile([P, 4 * 66], f32, name="accp", tag=f"accp{a}",
                             bufs=1)
                    for a in range(2)
                ]
                n_s = 4 * tb + 4  # s-tiles 0 .. 4*tb+3
                for i in range(n_s):
                    first = max(0, i - 4 * tb)  # first valid jj in block
                    sps = psp.tile([P, 1024], f32, name="sp", tag="sp",
                                   bufs=2)
                    # trim fully-masked leading columns when it helps:
                    # fp32r matmuls below N=256 run at 1/4 rate, so only
                    # slice when the remaining width stays >= 256.
                    c0 = P * first if 512 - P * first >= 256 else 0
                    for hh, pb in ((0, 0), (1, 64)):
                        nc.tensor.matmul(
                            sps[:, 512 * hh + c0 : 512 * hh + 512],
                            qkt_sb[(mk, i // 4)][
                                pb : pb + H, P * (i % 4) : P * (i % 4 + 1)
                            ],
                            qkt_sb[(mq, tb)][pb : pb + H, c0:512],
                            start=True,
                            stop=True,
                        )
                    et = e_p.tile([P, 1024], avdt, name="et", tag="et")
                    if first:
                        nc.scalar.activation(
                            et.rearrange("p (g c) -> p g c", g=2)[
                                :, :, P * first : 512
                            ],
                            sps.rearrange("p (g c) -> p g c", g=2)[
                                :, :, P * first : 512
                            ],
                            mybir.ActivationFunctionType.Exp,
                            scale=1.0 / math.sqrt(H),
                        )
                    else:
                        nc.scalar.activation(
                            et,
                            sps,
                            mybir.ActivationFunctionType.Exp,
                            scale=1.0 / math.sqrt(H),
                        )
                    dj = i - 4 * tb  # diagonal jj of this s-tile, if any
                    etd = None
                    if 0 <= dj <= 3:
                        # masked diagonal sub-tiles go to a separate tile so
                        # the non-diagonal AV matmuls don't serialize behind
                        # the mask write (tile-granular dependency tracking)
                        etd = e_p.tile([P, 2 * P], avdt, name="etd", tag="etd",
                                       bufs=2)
                        for hh in range(2):
                            nc.vector.tensor_mul(
                                etd[:, P * hh : P * (hh + 1)],
                                et[:, 512 * hh + P * dj : 512 * hh + P * (dj + 1)],
                                mask,
                            )
                    for jj in range(first, 4):
                        jglob = 4 * tb + jj
                        for hh in range(2):
                            if jj == dj:
                                lhs_e = etd[:, P * hh : P * (hh + 1)]
                            else:
                                lhs_e = et[
                                    :, 512 * hh + P * jj : 512 * hh + P * (jj + 1)
                                ]
                            # start=True clears has_written for the WHOLE
                            # psum bank: only the first group per bank
                            # issues it.
                            nc.tensor.matmul(
                                acc_t[hh][:, 66 * jj : 66 * jj + 66],
                                lhs_e,
                                v_sb[i][:, G * (h0 + hh) : G * (h0 + hh) + 66],
                                start=(i == 0 and jj == 0),
                                stop=(i == jglob),
                                skip_group_check=True,
                            )
                for jj in range(4):
                    for hh in range(2):
                        h = h0 + hh
                        s0 = 66 * jj
                        rec = small_p.tile([P, 1], f32, name="rec", tag="rec")
                        nc.vector.reciprocal(
                            rec, acc_t[hh][:, s0 + H : s0 + H + 1]
                        )
                        nc.vector.tensor_scalar_mul(
                            attn_t[(tb, hp)][
                                :, P * jj + H * hh : P * jj + H * (hh + 1)
                            ],
                            acc_t[hh][:, s0 : s0 + H],
                            rec,
                        )

        def projection(jb):
            """attn^T transposes + y = attn @ wp for 512-wide t-block jb."""
            attnT = {}
            for c in range(CD // P):
                pt = psp.tile([P, 512], mdt, name="atp", tag="wps")
                for ti in range(4):
                    nc.tensor.transpose(
                        pt[:, P * ti : P * (ti + 1)],
                        attn_t[(jb, c)][:, P * ti : P * (ti + 1)],
                        ident,
                    )
                at = e_p.tile([P, 512], mdt, name="at", tag="at", bufs=4)
                if jb == 3 and c == 0:
                    nc.scalar.copy(at, pt)
                else:
                    nc.vector.tensor_copy(at, pt)
                attnT[c] = at

            for jl in range(4):
                jt = 4 * jb + jl
                for n in range(2):
                    # block 3's projection runs after all attention: the
                    # score psum slots are free then
                    ps = psp.tile([P, 512], f32, name="yp",
                                  tag=("sp" if jb == 3 else "wps"))
                    for c in range(CD // P):
                        nc.tensor.matmul(
                            ps,
                            attnT[c][:, P * jl : P * (jl + 1)],
                            wp_sb[c][:, 512 * n : 512 * (n + 1)],
                            start=(c == 0),
                            stop=(c == CD // P - 1),
                        )
                    ysb = small_p.tile([P, 512], f32, name="ysb", tag="ysb",
                                       bufs=4)
                    if jb == 3 and (jl + n) % 2 == 0:
                        nc.scalar.copy(ysb, ps)
                    else:
                        nc.vector.tensor_copy(ysb, ps)
                    (nc.sync if (jb < 3 or n == 0) else nc.scalar).dma_start(
                        y_d[P * jt : P * (jt + 1), 512 * n : 512 * (n + 1)],
                        ysb,
                    )

        # emission order chosen so chunk-0 attention (ACT-bound) can overlap
        # the second half of phase 1 (PE-bound), and each chunk's projection
        # overlaps the next chunk's attention.
        phase1_block(0)
        attention(0)
        phase1_block(1)
        attention(1)
        phase1_block(2)
        attention(2)
        phase1_block(3)
        attention(3)
        projection(0)
        projection(1)
        projection(2)
        projection(3)

    nc.compile()
    return nc


def _get_module(mm_dt_name: str):
    if mm_dt_name not in _CACHE:
        _CACHE[mm_dt_name] = _build_module(mm_dt_name)
    return _CACHE[mm_dt_name]


def kernel(x, w_attn, b_attn, w_proj, b_proj, mm_dt_name: str = "float32r",
           trace: bool = False):
    from concourse.bass_utils import run_bass_kernel_spmd

    x = np.asarray(x, dtype=np.float32)
    w_attn = np.asarray(w_attn, dtype=np.float32)
    b_attn = np.asarray(b_attn, dtype=np.float32)
    w_proj = np.asarray(w_proj, dtype=np.float32)
    b_proj = np.asarray(b_proj, dtype=np.float32)

    nc = _get_module(mm_dt_name)

    import ml_dtypes

    avnp = np.dtype(ml_dtypes.bfloat16) if mm_dt_name == "float32r" else np.float32
    ident = np.eye(P, dtype=np.float32)
    mask = np.triu(np.ones((P, P), dtype=avnp))
    ones = np.ones((1, 512), dtype=np.float32)

    in_maps = []
    for core in range(N_CORES):
        b = core // 4
        g = core % 4
        c0 = CD * g
        wq = w_attn[:, c0 : c0 + CD]
        wk = w_attn[:, D + c0 : D + c0 + CD]
        wv = w_attn[:, 2 * D + c0 : 2 * D + c0 + CD]
        bq = b_attn[c0 : c0 + CD]
        bk = b_attn[D + c0 : D + c0 + CD]
        bvv = b_attn[2 * D + c0 : 2 * D + c0 + CD]
        in_maps.append(
            {
                "x": np.ascontiguousarray(x[b]),
                "wqk": np.ascontiguousarray(np.concatenate([wq, wk], axis=1)),
                "bqk": np.concatenate([bq, bk])[None, :].copy(),
                "wv": np.ascontiguousarray(wv),
                "bv": bvv[None, :].copy(),
                "wp": np.ascontiguousarray(w_proj[c0 : c0 + CD, :]),
                "ident": ident,
                "mask": mask,
                "ones": ones,
                "onescol": np.tile(np.array([1.0, 0.0], avnp), (P, HPC)),
            }
        )

    res = run_bass_kernel_spmd(
        nc, in_maps, core_ids=list(range(N_CORES)), trace=trace
    )

    out = np.zeros((B, T, D), dtype=np.float32)
    for core in range(N_CORES):
        out[core // 4] += res.results[core]["y"]
    out += b_proj[None, None, :]
    if trace:
        kernel.last_result = res
    return out
